# revision 1
# baseline (speedup 1.0000x reference)
"""BERT encoder (12 layers, B=8 T=512 D=768 H=12) on 8 Trainium2 NeuronCores.

Strategy: pure data parallelism — core b runs the full 12-layer stack for
batch element b. No collectives. All five per-layer GEMMs run on the tensor
engine in float32r (full-rate fp32); softmax uses ACT Exp with fused
row-sum accumulation; the softmax normalization is applied to P on DVE;
P^T comes from PE transpose-mode; layernorm runs fused on DVE/ACT.

Host-side folds (exact, negligible FLOPs):
  - attention scale 1/sqrt(dh) folded into Wq and bq
  - V bias folded through Wo1: b1 = bv @ Wo1 + bo1 (rows of softmax sum to 1)
  - weights pre-reshaped to the SBUF lhsT chunk layout
Zero biases / zero mask / identity LN affine (which is what
reference.setup_inputs() produces) skip their device ops entirely, but the
general paths are implemented and selected when inputs are nonzero.
"""

import numpy as np

L, B, T, D, H, DH = 12, 8, 512, 768, 12, 64
PD = 128
NKC = D // PD  # 6 contraction chunks
NTC = T // PD  # 4 token chunks
NG = 2         # N-groups per 768-wide output (384 each)
GW = D // NG   # 384
EPS = 1e-12
SCALE = 1.0 / np.sqrt(np.float32(DH))


def _split_excess_waits(nc, mybir, bass_rust, max_waits=1):
    """walrus codegen rejects instructions carrying more than a couple of
    sync waits; hoist excess waits onto same-engine NoOps placed before."""
    n = 0
    for f in nc.m.functions:
        for bb in f.blocks:
            new_insts = []
            changed = False
            for inst in bb.instructions:
                si = inst.sync_info
                if si is not None and len(si.on_wait) > max_waits:
                    waits = list(si.on_wait)
                    excess = waits[: len(waits) - max_waits]
                    for i in range(0, len(excess), max_waits):
                        chunk = excess[i : i + max_waits]
                        n += 1
                        nop = mybir.InstNoOp(
                            name=f"I-waitsplit-{n}", ins=[], outs=[]
                        )
                        nop.engine = inst.engine
                        nop.sync_info = bass_rust.SyncInfo(
                            on_wait=chunk, on_update=[]
                        )
                        new_insts.append(nop)
                        changed = True
                    si.on_wait = waits[len(waits) - max_waits :]
                new_insts.append(inst)
            if changed:
                bb.instructions[:] = new_insts
    return n


def build_nc(flags, split_waits=True):
    """Build the per-core Bass module. flags: dict of general-path toggles."""
    import concourse.bass as bass
    import concourse.tile as tile
    from concourse import mybir

    F32 = mybir.dt.float32
    F32R = mybir.dt.float32r
    AF = mybir.ActivationFunctionType
    OP = mybir.AluOpType

    use_mask = flags["use_mask"]
    use_bq = flags["use_bq"]
    use_bk = flags["use_bk"]
    use_b1 = flags["use_b1"]
    use_b2 = flags["use_b2"]
    use_ln1 = flags["use_ln1"]
    use_ln2 = flags["use_ln2"]

    nc = bass.Bass("TRN2", target_bir_lowering=False, debug=False)

    qs_d = nc.dram_tensor("qs", [T, D], F32R, kind="ExternalInput")
    hs_d = nc.dram_tensor("hs", [T, D], F32R, kind="ExternalInput")
    w_d = {
        name: nc.dram_tensor(name, [L, PD, NKC * D], F32R, kind="ExternalInput")
        for name in ("wq", "wk", "wv", "wo1", "wo2")
    }
    iden_d = nc.dram_tensor("iden", [PD, PD], F32R, kind="ExternalInput")
    bq_d = nc.dram_tensor("bq", [PD, L * NKC], F32, kind="ExternalInput") if use_bq else None
    bk_d = nc.dram_tensor("bk", [PD, L * NKC], F32, kind="ExternalInput") if use_bk else None
    mask_d = nc.dram_tensor("mask", [PD, NTC], F32, kind="ExternalInput") if use_mask else None
    sela_d = nc.dram_tensor("sela", [1, PD], F32R, kind="ExternalInput")
    selb_d = nc.dram_tensor("selb", [1, PD], F32R, kind="ExternalInput")
    vones_d = nc.dram_tensor("vones", [PD, H], F32R, kind="ExternalInput")
    b1_d = nc.dram_tensor("b1bc", [L, PD, D], F32, kind="ExternalInput") if use_b1 else None
    b2_d = nc.dram_tensor("b2bc", [L, PD, D], F32, kind="ExternalInput") if use_b2 else None
    ln1w_d = nc.dram_tensor("ln1wbc", [L, PD, D], F32, kind="ExternalInput") if use_ln1 else None
    ln1b_d = nc.dram_tensor("ln1bbc", [L, PD, D], F32, kind="ExternalInput") if use_ln1 else None
    ln2w_d = nc.dram_tensor("ln2wbc", [L, PD, D], F32, kind="ExternalInput") if use_ln2 else None
    ln2b_d = nc.dram_tensor("ln2bbc", [L, PD, D], F32, kind="ExternalInput") if use_ln2 else None
    out_d = nc.dram_tensor("out", [T, D], F32R, kind="ExternalOutput")

    evac_ctr = [0]

    with tile.TileContext(nc) as tc:
        import contextlib

        with contextlib.ExitStack() as ctx:
            p_w = ctx.enter_context(tc.tile_pool(name="w", bufs=3))
            p_qt = ctx.enter_context(tc.tile_pool(name="qt", bufs=6))
            p_hid = ctx.enter_context(tc.tile_pool(name="hid", bufs=8))
            p_ht = ctx.enter_context(tc.tile_pool(name="ht", bufs=6))
            p_act = ctx.enter_context(tc.tile_pool(name="act", bufs=12))
            p_ctx = ctx.enter_context(tc.tile_pool(name="ctxp", bufs=7))
            p_v = ctx.enter_context(tc.tile_pool(name="v", bufs=4))
            p_pt = ctx.enter_context(tc.tile_pool(name="pt", bufs=8))
            p_r = ctx.enter_context(tc.tile_pool(name="r", bufs=2))
            p_z = ctx.enter_context(tc.tile_pool(name="z", bufs=2))
            p_sm = ctx.enter_context(tc.tile_pool(name="sm", bufs=2))
            p_c1 = ctx.enter_context(tc.tile_pool(name="c1", bufs=1))
            p_bc = ctx.enter_context(tc.tile_pool(name="bc", bufs=2))
            ps_a = ctx.enter_context(tc.tile_pool(name="psA", bufs=3, space="PSUM"))
            ps_b = ctx.enter_context(tc.tile_pool(name="psB", bufs=2, space="PSUM"))
            ps_c = ctx.enter_context(tc.tile_pool(name="psC", bufs=3, space="PSUM"))

            def evac(dst_ap, src_ap):
                """PSUM -> SBUF copy, alternating ACT/DVE to balance load."""
                evac_ctr[0] += 1
                if evac_ctr[0] % 2 == 0:
                    nc.scalar.copy(dst_ap, src_ap)
                else:
                    nc.vector.tensor_copy(dst_ap, src_ap)

            # ---- one-time constants / inputs ----
            iden = p_c1.tile([PD, PD], F32R, tag="iden")
            nc.sync.dma_start(iden[:], iden_d.ap())
            if use_bq:
                bq_t = p_c1.tile([PD, L * NKC], F32, tag="bq")
                nc.sync.dma_start(bq_t[:], bq_d.ap())
            if use_bk:
                bk_t = p_c1.tile([PD, L * NKC], F32, tag="bk")
                nc.sync.dma_start(bk_t[:], bk_d.ap())
            if use_mask:
                mask_t = p_c1.tile([PD, NTC], F32, tag="mask")
                nc.sync.dma_start(mask_t[:], mask_d.ap())
            sela_t = p_c1.tile([1, PD], F32R, tag="sela")
            nc.sync.dma_start(sela_t[:], sela_d.ap())
            selb_t = p_c1.tile([1, PD], F32R, tag="selb")
            nc.sync.dma_start(selb_t[:], selb_d.ap())
            vones_t = p_c1.tile([PD, H], F32R, tag="vones")
            nc.sync.dma_start(vones_t[:], vones_d.ap())

            qs_n = []
            for tc_i in range(NTC):
                t = p_hid.tile([PD, D], F32R, tag="hid")
                nc.sync.dma_start(t[:], qs_d.ap()[tc_i * PD : (tc_i + 1) * PD, :])
                qs_n.append(t)
            h_tiles = []
            for tc_i in range(NTC):
                t = p_hid.tile([PD, D], F32R, tag="hid")
                nc.sync.dma_start(t[:], hs_d.ap()[tc_i * PD : (tc_i + 1) * PD, :])
                h_tiles.append(t)

            def transpose_norm_to_T(src_tiles, pool, tag):
                """[T, D] (4x[128,768] f32r) -> [D, T] (6x[128,512] f32r)."""
                out = []
                for kc in range(NKC):
                    pt = ps_b.tile([PD, T], F32R, tag="pb")
                    for tc_i in range(NTC):
                        nc.tensor.transpose(
                            pt[:, tc_i * PD : (tc_i + 1) * PD],
                            src_tiles[tc_i][:, kc * PD : (kc + 1) * PD],
                            iden[:],
                        )
                    dst = pool.tile([PD, T], F32R, tag=tag)
                    evac(dst[:], pt[:])
                    out.append(dst)
                return out

            qT = transpose_norm_to_T(qs_n, p_qt, "qt")

            # ---- layers ----
            for l in range(L):
                wq_t = p_w.tile([PD, NKC * D], F32R, tag="w")
                nc.sync.dma_start(wq_t[:], w_d["wq"].ap()[l])
                wk_t = p_w.tile([PD, NKC * D], F32R, tag="w")
                nc.sync.dma_start(wk_t[:], w_d["wk"].ap()[l])
                wv_t = p_w.tile([PD, NKC * D], F32R, tag="w")
                nc.sync.dma_start(wv_t[:], w_d["wv"].ap()[l])

                hT = transpose_norm_to_T(h_tiles, p_ht, "ht")

                # Q^T, K^T: [D, T], d_out on partitions
                def proj_T(w_tile, rhs_tiles, bias_t, use_bias):
                    outs = []
                    for mc in range(NKC):
                        pp = ps_a.tile([PD, T], F32, tag="pa")
                        for kc in range(NKC):
                            nc.tensor.matmul(
                                pp[:],
                                w_tile[:, kc * D + mc * PD : kc * D + (mc + 1) * PD],
                                rhs_tiles[kc][:],
                                start=(kc == 0),
                                stop=(kc == NKC - 1),
                            )
                        dst = p_act.tile([PD, T], F32R, tag="qk")
                        if use_bias:
                            nc.scalar.activation(
                                dst[:], pp[:], AF.Identity,
                                bias=bias_t[:, l * NKC + mc : l * NKC + mc + 1],
                                scale=1.0,
                            )
                        else:
                            evac(dst[:], pp[:])
                        outs.append(dst)
                    return outs

                QT = proj_T(wq_t, qT, bq_t if use_bq else None, use_bq)
                KT = proj_T(wk_t, hT, bk_t if use_bk else None, use_bk)

                # V: augmented normal layout [k, 12*65]; head h at cols
                # 65h..65h+63, ones at col 65h+64 (emits the softmax
                # denominator as row 64 of the PV product).
                V = []
                for tc_i in range(NTC):
                    vt = p_v.tile([PD, H * 65], F32R, tag="v")
                    for ng in range(NG):
                        pp = ps_b.tile([PD, GW], F32, tag="pb")
                        for kc in range(NKC):
                            nc.tensor.matmul(
                                pp[:],
                                hT[kc][:, tc_i * PD : (tc_i + 1) * PD],
                                wv_t[:, kc * D + ng * GW : kc * D + (ng + 1) * GW],
                                start=(kc == 0),
                                stop=(kc == NKC - 1),
                            )
                        dst = vt[:, ng * 390 : (ng + 1) * 390].rearrange(
                            "p (h c) -> p h c", c=65
                        )[:, :, 0:64]
                        src_ = pp[:].rearrange("p (h c) -> p h c", c=64)
                        evac(dst, src_)
                    ones_dst = vt[:].rearrange("p (h c) -> p h c", c=65)[:, :, 64:65]
                    nc.vector.tensor_copy(
                        ones_dst, vones_t[:].rearrange("p (h o) -> p h o", o=1)
                    )
                    V.append(vt)

                wo1_t = p_w.tile([PD, NKC * D], F32R, tag="w")
                nc.sync.dma_start(wo1_t[:], w_d["wo1"].ap()[l])
                wo2_t = p_w.tile([PD, NKC * D], F32R, tag="w")
                nc.sync.dma_start(wo2_t[:], w_d["wo2"].ap()[l])

                ctxT = [
                    p_ctx.tile([PD, T], F32R, tag="ctx", name=f"ctx{i}")
                    for i in range(NKC)
                ]

                for pair in range(H // 2):
                    h0, h1 = pair * 2, pair * 2 + 1
                    qtile = QT[pair]
                    ktile = KT[pair]
                    # both heads' score matmuls first, so the ACT exp
                    # pipeline runs ahead of the PV accumulation chain
                    sps = {}
                    pts = {}
                    for sub in range(2):
                        hh = pair * 2 + sub
                        off = 64 * sub
                        for kb in range(NTC):
                            sp = ps_a.tile([PD, T], F32, tag="pa", name=f"sp{hh}_{kb}")
                            nc.tensor.matmul(
                                sp[:],
                                ktile[off : off + 64, kb * PD : (kb + 1) * PD],
                                qtile[off : off + 64, :],
                                start=True,
                                stop=True,
                            )
                            sps[(sub, kb)] = sp
                            pt = p_pt.tile([PD, T], F32R, tag="pts",
                                           name=f"pt{hh}_{kb}")
                            if use_mask:
                                nc.scalar.activation(
                                    pt[:], sp[:], AF.Exp,
                                    bias=mask_t[:, kb : kb + 1], scale=1.0,
                                )
                            else:
                                nc.scalar.activation(
                                    pt[:], sp[:], AF.Exp, bias=0.0, scale=1.0,
                                )
                            pts[(sub, kb)] = pt
                    cps = []
                    dens = []
                    for sub in range(2):
                        hh = pair * 2 + sub
                        cp = ps_c.tile([65, T], F32, tag="ctxp", name=f"cp{hh}")
                        for kb in range(NTC):
                            nc.tensor.matmul(
                                cp[:],
                                V[kb][:, 65 * hh : 65 * hh + 65],
                                pts[(sub, kb)][:],
                                start=(kb == 0),
                                stop=(kb == NTC - 1),
                            )
                        # raw denominator row -> SBUF (ACT, off the DVE path)
                        den = p_sm.tile([1, T], F32R, tag="den", bufs=4,
                                        name=f"den{hh}")
                        nc.scalar.copy(den[:], cp[64:65, :])
                        dens.append(den)
                        cps.append((hh, cp))
                    # R_raw = rows 0-63 <- den0, rows 64-127 <- den1 (PE
                    # outer products), then one full-width reciprocal
                    pr = ps_b.tile([PD, T], F32, tag="pb", name=f"pr{pair}")
                    nc.tensor.matmul(
                        pr[:], sela_t[:], dens[0][:], start=True, stop=False
                    )
                    nc.tensor.matmul(
                        pr[:], selb_t[:], dens[1][:], start=False, stop=True
                    )
                    rsb = p_r.tile([PD, T], F32, tag="rsb", name=f"r{pair}")
                    nc.vector.reciprocal(rsb[:], pr[:])
                    for hh, cp in cps:
                        off = 64 * (hh % 2)
                        nc.vector.tensor_tensor(
                            ctxT[hh // 2][off : off + 64, :],
                            cp[0:64, :],
                            rsb[off : off + 64, :],
                            op=OP.mult,
                        )

                # ---- output block: z = x @ W + residual, then LN ----
                def out_block(lhsT_tiles, w_tile, res_tiles, badd_d, use_badd,
                              lnw_d_, lnb_d_, use_ln, out_tag, is_last):
                    outs = []
                    if use_badd:
                        badd_t = p_bc.tile([PD, D], F32, tag="badd")
                        nc.sync.dma_start(badd_t[:], badd_d.ap()[l])
                    if use_ln:
                        lnw_t = p_bc.tile([PD, D], F32, tag="lnw")
                        nc.sync.dma_start(lnw_t[:], lnw_d_.ap()[l])
                        lnb_t = p_bc.tile([PD, D], F32, tag="lnb")
                        nc.sync.dma_start(lnb_t[:], lnb_d_.ap()[l])
                    for tc_i in range(NTC):
                        z = p_z.tile([PD, D], F32, tag="z")
                        s01 = p_sm.tile([PD, NG], F32, tag="s01")
                        for ng in range(NG):
                            pp = ps_b.tile([PD, GW], F32, tag="pb")
                            for kc in range(NKC):
                                nc.tensor.matmul(
                                    pp[:],
                                    lhsT_tiles[kc][:, tc_i * PD : (tc_i + 1) * PD],
                                    w_tile[:, kc * D + ng * GW : kc * D + (ng + 1) * GW],
                                    start=(kc == 0),
                                    stop=(kc == NKC - 1),
                                )
                            sl = slice(ng * GW, (ng + 1) * GW)
                            if use_badd:
                                nc.vector.scalar_tensor_tensor(
                                    z[:, sl], pp[:], 1.0, res_tiles[tc_i][:, sl],
                                    op0=OP.mult, op1=OP.add,
                                )
                                nc.vector.scalar_tensor_tensor(
                                    z[:, sl], z[:, sl], 1.0, badd_t[:, sl],
                                    op0=OP.mult, op1=OP.add,
                                    accum_out=s01[:, ng : ng + 1],
                                )
                            else:
                                nc.vector.scalar_tensor_tensor(
                                    z[:, sl], pp[:], 1.0, res_tiles[tc_i][:, sl],
                                    op0=OP.mult, op1=OP.add,
                                    accum_out=s01[:, ng : ng + 1],
                                )
                        # layernorm over the full 768-wide row
                        ssum = p_sm.tile([PD, 1], F32, tag="ssum")
                        nc.vector.tensor_tensor(
                            ssum[:], s01[:, 0:1], s01[:, 1:2], op=OP.add
                        )
                        uneg = p_sm.tile([PD, 1], F32, tag="uneg")
                        nc.vector.tensor_scalar_mul(uneg[:], ssum[:], -1.0 / D)
                        sq = p_z.tile([PD, D], F32, tag="sq")
                        ssq = p_sm.tile([PD, 1], F32, tag="ssq")
                        nc.scalar.activation(
                            sq[:], z[:], AF.Square, bias=uneg[:], scale=1.0,
                            accum_out=ssq[:],
                        )
                        var_eps = p_sm.tile([PD, 1], F32, tag="vareps")
                        nc.vector.tensor_scalar(
                            var_eps[:], ssq[:], 1.0 / D, EPS, op0=OP.mult, op1=OP.add
                        )
                        stdev = p_sm.tile([PD, 1], F32, tag="stdev")
                        nc.scalar.sqrt(stdev[:], var_eps[:])
                        rstd = p_sm.tile([PD, 1], F32, tag="rstd")
                        nc.vector.reciprocal(rstd[:], stdev[:])
                        urneg = p_sm.tile([PD, 1], F32, tag="urneg")
                        nc.vector.tensor_tensor(
                            urneg[:], uneg[:], rstd[:], op=OP.mult
                        )
                        o = p_hid.tile([PD, D], F32R, tag=out_tag)
                        if use_ln:
                            on = p_z.tile([PD, D], F32, tag="sq")
                            nc.vector.tensor_scalar(
                                on[:], z[:], rstd[:], urneg[:], op0=OP.mult, op1=OP.add
                            )
                            nc.vector.tensor_tensor(
                                on[:], on[:], lnw_t[:], op=OP.mult
                            )
                            nc.vector.tensor_tensor(
                                o[:], on[:], lnb_t[:], op=OP.add
                            )
                        else:
                            nc.vector.tensor_scalar(
                                o[:], z[:], rstd[:], urneg[:], op0=OP.mult, op1=OP.add
                            )
                        if is_last:
                            nc.sync.dma_start(
                                out_d.ap()[tc_i * PD : (tc_i + 1) * PD, :], o[:]
                            )
                        outs.append(o)
                    return outs

                a_tiles = out_block(
                    ctxT, wo1_t, h_tiles, b1_d, use_b1,
                    ln1w_d, ln1b_d, use_ln1, "hid", False,
                )
                aT = transpose_norm_to_T(a_tiles, p_ht, "ht")
                h_tiles = out_block(
                    aT, wo2_t, a_tiles, b2_d, use_b2,
                    ln2w_d, ln2b_d, use_ln2, "hid", l == L - 1,
                )

    if split_waits:
        import bass_rust

        _split_excess_waits(nc, mybir, bass_rust)
    return nc


def prep_inputs(inputs):
    """Host-side folds. Returns (flags, per-core-invariant map, per-core list)."""
    g = {k: np.asarray(v, dtype=np.float32) for k, v in inputs.items()}

    wq_s = g["Wq"] * SCALE
    bq_s = g["bq"] * SCALE
    b1 = np.einsum("ld,ldo->lo", g["bv"], g["Wo1"]) + g["bo1"]
    b2 = g["bo2"]

    flags = {
        "use_mask": bool(np.any(g["attention_mask"])),
        "use_bq": bool(np.any(bq_s)),
        "use_bk": bool(np.any(g["bk"])),
        "use_b1": bool(np.any(b1)),
        "use_b2": bool(np.any(b2)),
        "use_ln1": bool(np.any(g["ln1_w"] != 1.0) or np.any(g["ln1_b"])),
        "use_ln2": bool(np.any(g["ln2_w"] != 1.0) or np.any(g["ln2_b"])),
    }

    def wfmt(w):
        return np.ascontiguousarray(
            w.reshape(L, NKC, PD, D).transpose(0, 2, 1, 3).reshape(L, PD, NKC * D)
        )

    def bfmt(b):
        return np.ascontiguousarray(
            b.reshape(L, NKC, PD).transpose(2, 0, 1).reshape(PD, L * NKC)
        )

    shared = {
        "wq": wfmt(wq_s),
        "wk": wfmt(g["Wk"]),
        "wv": wfmt(g["Wv"]),
        "wo1": wfmt(g["Wo1"]),
        "wo2": wfmt(g["Wo2"]),
        "iden": np.eye(PD, dtype=np.float32),
    }
    if flags["use_bq"]:
        shared["bq"] = bfmt(bq_s)
    if flags["use_bk"]:
        shared["bk"] = bfmt(g["bk"])
    sela = np.zeros((1, PD), dtype=np.float32)
    sela[0, :64] = 1.0
    selb = np.zeros((1, PD), dtype=np.float32)
    selb[0, 64:] = 1.0
    shared["sela"] = sela
    shared["selb"] = selb
    shared["vones"] = np.ones((PD, H), dtype=np.float32)
    if flags["use_b1"]:
        shared["b1bc"] = np.ascontiguousarray(
            np.broadcast_to(b1[:, None, :], (L, PD, D))
        )
    if flags["use_b2"]:
        shared["b2bc"] = np.ascontiguousarray(
            np.broadcast_to(b2[:, None, :], (L, PD, D))
        )
    if flags["use_ln1"]:
        shared["ln1wbc"] = np.ascontiguousarray(
            np.broadcast_to(g["ln1_w"][:, None, :], (L, PD, D))
        )
        shared["ln1bbc"] = np.ascontiguousarray(
            np.broadcast_to(g["ln1_b"][:, None, :], (L, PD, D))
        )
    if flags["use_ln2"]:
        shared["ln2wbc"] = np.ascontiguousarray(
            np.broadcast_to(g["ln2_w"][:, None, :], (L, PD, D))
        )
        shared["ln2bbc"] = np.ascontiguousarray(
            np.broadcast_to(g["ln2_b"][:, None, :], (L, PD, D))
        )

    per_core = []
    for b in range(B):
        m = dict(shared)
        m["qs"] = np.ascontiguousarray(g["query_states"][b])
        m["hs"] = np.ascontiguousarray(g["hidden_states"][b])
        if flags["use_mask"]:
            m["mask"] = np.ascontiguousarray(
                g["attention_mask"][b].reshape(NTC, PD).T
            )
        per_core.append(m)
    return flags, per_core


TRACE = False
LAST_EXEC_NS = None
LAST_RESULTS = None


def kernel(**inputs):
    global LAST_EXEC_NS, LAST_RESULTS
    from concourse.bass_utils import run_bass_kernel_spmd

    flags, per_core = prep_inputs(inputs)
    nc = build_nc(flags)
    kw = {}
    if TRACE:
        kw = dict(trace=True, tmpdir="/root/problem/trace_out")
        import os

        os.makedirs("/root/problem/trace_out", exist_ok=True)
    res = run_bass_kernel_spmd(nc, per_core, core_ids=list(range(B)), **kw)
    LAST_EXEC_NS = res.exec_time_ns
    LAST_RESULTS = res
    out = np.stack([np.asarray(res.results[b]["out"]) for b in range(B)], axis=0)
    return out.astype(np.float32)



# revision 10
# speedup vs baseline: 1.1384x; 1.1384x over previous
"""BERT encoder (12 layers, B=8 T=512 D=768 H=12) on 8 Trainium2 NeuronCores.

Strategy: pure data parallelism — core b runs the full 12-layer stack for
batch element b. No collectives.

Tensor-engine budget is the bottleneck, and matmul cost on TRN2 is
(output free size) x (cycles/row) independent of K and M, so the kernel
maximizes contraction per instruction: all five per-layer GEMMs and the
PV product run in fp8e4m3 with DoubleRow perf mode (two 128-deep k-tiles
per instruction, 0.5 cycles/row -> 4x fewer PE column-cycles than f32r).
QK^T scores stay float32r (K=64 can't exploit DoubleRow without a costly
re-fold). Softmax: ACT Exp with the 1/sqrt(dh) scale and mask folded in,
emitting P^T directly in fp8; the denominator comes out of the PV matmul
via an appended ones-column in V, is reciprocal'd on a [2,T] tile and
broadcast to 128 partitions by a single K=2 PE outer product.

fp8 scaling (all powers of two, exact): weights x64; Q/K/V descaled by
1/64 at PSUM evac; ctx scaled x64 by the denominator broadcast; the
output GEMM descales 1/4096 (ctx x wo1) or 1/64 (a x wo2) in the fused
residual-add. Activations (std ~1) cast straight to fp8.

Engine placement: exp/sqrt/K-evac on ACT; residual stt, divides, LN
finals, Q-evac on DVE; transpose evacs, V evac, denominator copies, LN
variance (sum z^2 via stt-accum) on the otherwise-idle Pool engine.

Host-side folds (exact, negligible FLOPs):
  - V bias folded through Wo1: b1 = bv @ Wo1 + bo1 (rows of softmax sum to 1)
  - weights pre-reshaped to the DoubleRow lhsT chunk layout
Zero biases / zero mask / identity LN affine (which is what
reference.setup_inputs() produces) skip their device ops entirely, but the
general paths are implemented and selected when inputs are nonzero.
"""

import numpy as np

L, B, T, D, H, DH = 12, 8, 512, 768, 12, 64
PD = 128
NKC = D // PD   # 6 contraction chunks of 128
NKP = NKC // 2  # 3 DoubleRow chunk-pairs of 256
NTC = T // PD   # 4 token chunks
NG = 2          # N-groups per 768-wide output (384 each)
GW = D // NG    # 384
EPS = 1e-12
SCALE = 1.0 / np.sqrt(np.float32(DH))
WS = 64.0       # fp8 weight scale (power of two, exact)
FP8_MAX = 240.0


def _split_excess_waits(nc, mybir, bass_rust, max_waits=1):
    """walrus codegen rejects instructions carrying more than a couple of
    sync waits; hoist excess waits onto same-engine NoOps placed before."""
    n = 0
    for f in nc.m.functions:
        for bb in f.blocks:
            new_insts = []
            changed = False
            for inst in bb.instructions:
                si = inst.sync_info
                if si is not None and len(si.on_wait) > max_waits:
                    waits = list(si.on_wait)
                    excess = waits[: len(waits) - max_waits]
                    for i in range(0, len(excess), max_waits):
                        chunk = excess[i : i + max_waits]
                        n += 1
                        nop = mybir.InstNoOp(
                            name=f"I-waitsplit-{n}", ins=[], outs=[]
                        )
                        nop.engine = inst.engine
                        nop.sync_info = bass_rust.SyncInfo(
                            on_wait=chunk, on_update=[]
                        )
                        new_insts.append(nop)
                        changed = True
                    si.on_wait = waits[len(waits) - max_waits :]
                new_insts.append(inst)
            if changed:
                bb.instructions[:] = new_insts
    return n


def build_nc(flags, split_waits=True):
    """Build the per-core Bass module. flags: dict of general-path toggles."""
    import concourse.bass as bass
    import concourse.tile as tile
    from concourse import mybir

    F32 = mybir.dt.float32
    F32R = mybir.dt.float32r
    FP8 = mybir.dt.float8e4
    AF = mybir.ActivationFunctionType
    OP = mybir.AluOpType
    DR = mybir.MatmulPerfMode.DoubleRow

    use_mask = flags["use_mask"]
    use_bq = flags["use_bq"]
    use_bk = flags["use_bk"]
    use_b1 = flags["use_b1"]
    use_b2 = flags["use_b2"]
    use_ln1 = flags["use_ln1"]
    use_ln2 = flags["use_ln2"]

    nc = bass.Bass("TRN2", target_bir_lowering=False, debug=False)

    qs_d = nc.dram_tensor("qs", [T, D], F32R, kind="ExternalInput")
    hs_d = nc.dram_tensor("hs", [T, D], F32R, kind="ExternalInput")
    BF16 = mybir.dt.bfloat16
    w_d = {
        name: nc.dram_tensor(name, [L, PD, NKC * D], FP8, kind="ExternalInput")
        for name in ("wq", "wk", "wv", "wo1")
    }
    w_d["wo2"] = nc.dram_tensor("wo2", [L, PD, NKC * D], BF16, kind="ExternalInput")
    iden_d = nc.dram_tensor("iden", [PD, PD], F32R, kind="ExternalInput")
    bq_d = nc.dram_tensor("bq", [PD, L * NKC], F32, kind="ExternalInput") if use_bq else None
    bk_d = nc.dram_tensor("bk", [PD, L * NKC], F32, kind="ExternalInput") if use_bk else None
    mask_d = nc.dram_tensor("mask", [PD, NTC], F32, kind="ExternalInput") if use_mask else None
    sel2_d = nc.dram_tensor("sel2", [2, PD], F32R, kind="ExternalInput")
    b1_d = nc.dram_tensor("b1bc", [L, PD, D], F32, kind="ExternalInput") if use_b1 else None
    b2_d = nc.dram_tensor("b2bc", [L, PD, D], F32, kind="ExternalInput") if use_b2 else None
    ln1w_d = nc.dram_tensor("ln1wbc", [L, PD, D], F32, kind="ExternalInput") if use_ln1 else None
    ln1b_d = nc.dram_tensor("ln1bbc", [L, PD, D], F32, kind="ExternalInput") if use_ln1 else None
    ln2w_d = nc.dram_tensor("ln2wbc", [L, PD, D], F32, kind="ExternalInput") if use_ln2 else None
    ln2b_d = nc.dram_tensor("ln2bbc", [L, PD, D], F32, kind="ExternalInput") if use_ln2 else None
    out_d = nc.dram_tensor("out", [T, D], F32R, kind="ExternalOutput")

    def w8ap(t, i, sl):
        """[128, NKC*D] fp8 weight tile -> [128, 2, sl] DoubleRow slice for
        chunk-pair i."""
        return t[:].rearrange("p (i two d) -> p i two d", two=2, d=D)[:, i, :, sl]

    with tile.TileContext(nc) as tc:
        import contextlib

        with contextlib.ExitStack() as ctx:
            p_w = ctx.enter_context(tc.tile_pool(name="w", bufs=4))
            p_qt8 = ctx.enter_context(tc.tile_pool(name="qt8", bufs=3))
            p_hid = ctx.enter_context(tc.tile_pool(name="hid", bufs=8))
            p_ht8 = ctx.enter_context(tc.tile_pool(name="ht8", bufs=7))
            p_qk = ctx.enter_context(tc.tile_pool(name="qk", bufs=14))
            p_ctx8 = ctx.enter_context(tc.tile_pool(name="ctx8", bufs=4))
            p_v8 = ctx.enter_context(tc.tile_pool(name="v8", bufs=3))
            p_pt8 = ctx.enter_context(tc.tile_pool(name="pt8", bufs=9))
            p_den = ctx.enter_context(tc.tile_pool(name="den", bufs=4))
            p_z = ctx.enter_context(tc.tile_pool(name="z", bufs=4))
            p_sm = ctx.enter_context(tc.tile_pool(name="sm", bufs=2))
            p_c1 = ctx.enter_context(tc.tile_pool(name="c1", bufs=1))
            p_bc = ctx.enter_context(tc.tile_pool(name="bc", bufs=2))
            ps_a = ctx.enter_context(tc.tile_pool(name="psA", bufs=3, space="PSUM"))
            ps_b = ctx.enter_context(tc.tile_pool(name="psB", bufs=2, space="PSUM"))
            ps_c = ctx.enter_context(tc.tile_pool(name="psC", bufs=3, space="PSUM"))

            # ---- one-time constants / inputs ----
            iden = p_c1.tile([PD, PD], F32R, tag="iden")
            nc.sync.dma_start(iden[:], iden_d.ap())
            if use_bq:
                bq_t = p_c1.tile([PD, L * NKC], F32, tag="bq")
                nc.sync.dma_start(bq_t[:], bq_d.ap())
            if use_bk:
                bk_t = p_c1.tile([PD, L * NKC], F32, tag="bk")
                nc.sync.dma_start(bk_t[:], bk_d.ap())
            if use_mask:
                mask_t = p_c1.tile([PD, NTC], F32, tag="mask")
                nc.sync.dma_start(mask_t[:], mask_d.ap())
            sel2_t = p_c1.tile([2, PD], F32R, tag="sel2")
            nc.sync.dma_start(sel2_t[:], sel2_d.ap())

            qs_n = []
            for tc_i in range(NTC):
                t = p_hid.tile([PD, D], F32R, tag="hid")
                nc.sync.dma_start(t[:], qs_d.ap()[tc_i * PD : (tc_i + 1) * PD, :])
                qs_n.append(t)
            h_tiles = []
            for tc_i in range(NTC):
                t = p_hid.tile([PD, D], F32R, tag="hid")
                nc.sync.dma_start(t[:], hs_d.ap()[tc_i * PD : (tc_i + 1) * PD, :])
                h_tiles.append(t)

            def transpose_to(src_tiles, pool, tag, dt, pair):
                """[T, D] (4x[128,768] f32r) -> [D, T], transposed on PE
                (f32r) and cast at evac on Pool. pair=True packs DoubleRow
                chunk-pairs (NKP tiles of [128, 2, T]); else NKC flat tiles
                of [128, T]."""
                out = []
                for i in range(NKP if pair else NKC):
                    dst = pool.tile([PD, 2, T] if pair else [PD, T], dt, tag=tag)
                    for j in range(2 if pair else 1):
                        kc = 2 * i + j if pair else i
                        pt = ps_b.tile([PD, T], F32R, tag="pb")
                        for tc_i in range(NTC):
                            nc.tensor.transpose(
                                pt[:, tc_i * PD : (tc_i + 1) * PD],
                                src_tiles[tc_i][:, kc * PD : (kc + 1) * PD],
                                iden[:],
                            )
                        nc.vector.tensor_copy(dst[:, j, :] if pair else dst[:], pt[:])
                    out.append(dst)
                return out

            qT8 = transpose_to(qs_n, p_qt8, "qt8", FP8, True)

            # ---- layers ----
            for l in range(L):
                wq_t = p_w.tile([PD, NKC * D], FP8, tag="w")
                nc.sync.dma_start(wq_t[:], w_d["wq"].ap()[l])
                wk_t = p_w.tile([PD, NKC * D], FP8, tag="w")
                nc.sync.dma_start(wk_t[:], w_d["wk"].ap()[l])
                wv_t = p_w.tile([PD, NKC * D], FP8, tag="w")
                nc.sync.dma_start(wv_t[:], w_d["wv"].ap()[l])

                hT8 = transpose_to(h_tiles, p_ht8, "ht8", FP8, True)

                # Q^T, K^T: [D, T] f32r, d_out on partitions. DoubleRow over
                # the 768-deep contraction (3 chained K=256 matmuls).
                def proj_T(w_tile, rhs8, bias_t, use_bias, on_act):
                    outs = []
                    for mc in range(NKC):
                        pp = ps_a.tile([PD, T], F32, tag="pa")
                        sl = slice(mc * PD, (mc + 1) * PD)
                        for i in range(NKP):
                            nc.tensor.matmul(
                                pp[:],
                                w8ap(w_tile, i, sl),
                                rhs8[i][:],
                                start=(i == 0),
                                stop=(i == NKP - 1),
                                perf_mode=DR,
                            )
                        dst = p_qk.tile([PD, T], F32R, tag="qk")
                        bias_ap = (
                            bias_t[:, l * NKC + mc : l * NKC + mc + 1]
                            if use_bias
                            else 0.0
                        )
                        if on_act:
                            nc.scalar.activation(
                                dst[:], pp[:], AF.Identity,
                                bias=bias_ap, scale=1.0 / WS,
                            )
                        else:
                            nc.vector.tensor_scalar(
                                dst[:], pp[:], 1.0 / WS, bias_ap,
                                op0=OP.mult, op1=OP.add,
                            )
                        outs.append(dst)
                    return outs

                QT = proj_T(wq_t, qT8, bq_t if use_bq else None, use_bq, False)
                KT = proj_T(wk_t, hT8, bk_t if use_bk else None, use_bk, True)

                # V: fp8, augmented layout per kb-pair: [128, 2, H*128];
                # head h at cols 128h..128h+63; a ones column at 128h+64
                # (h even) or 128h+65 (h odd), zeros elsewhere, so the PV
                # product of a head pair emits the two softmax denominators
                # in the partition-aligned rows 64:66. The pad to a 128
                # stride keeps dual-fp8 LDWEIGHTS legal (M must be 128) and
                # costs nothing: matmul time only scales with N.
                V8 = []
                for tp in range(NTC // 2):
                    vt = p_v8.tile([PD, 2, H * PD], FP8, tag="v8")
                    for j in range(2):
                        tc_i = 2 * tp + j
                        pairs = vt[:, j, :].rearrange("p (m c) -> p m c", c=256)
                        nc.gpsimd.memset(pairs[:, :, 64:128], 0.0)
                        nc.gpsimd.memset(pairs[:, :, 192:256], 0.0)
                        nc.gpsimd.memset(pairs[:, :, 64:65], 1.0)
                        nc.gpsimd.memset(pairs[:, :, 193:194], 1.0)
                        for ng in range(NG):
                            pp = ps_b.tile([PD, GW], F32, tag="pb")
                            sl = slice(ng * GW, (ng + 1) * GW)
                            for i in range(NKP):
                                nc.tensor.matmul(
                                    pp[:],
                                    hT8[i][:, :, tc_i * PD : (tc_i + 1) * PD],
                                    w8ap(wv_t, i, sl),
                                    start=(i == 0),
                                    stop=(i == NKP - 1),
                                    perf_mode=DR,
                                )
                            dst = vt[:, j, ng * 6 * PD : (ng + 1) * 6 * PD].rearrange(
                                "p (h c) -> p h c", c=PD
                            )[:, :, 0:64]
                            src_ = pp[:].rearrange("p (h c) -> p h c", c=64)
                            nc.vector.tensor_scalar_mul(dst, src_, 1.0 / WS)
                    V8.append(vt)

                wo1_t = p_w.tile([PD, NKC * D], FP8, tag="w")
                nc.sync.dma_start(wo1_t[:], w_d["wo1"].ap()[l])
                wo2_t = p_w.tile([PD, NKC * D], BF16, tag="wbf")
                nc.sync.dma_start(wo2_t[:], w_d["wo2"].ap()[l])

                # ctx^T in fp8 DoubleRow pair layout: NKP tiles [128, 2, T].
                # head h lives in tile h//4, sub (h//2)%2, rows 64*(h%2).
                ctxT8 = [
                    p_ctx8.tile([PD, 2, T], FP8, tag="ctx8", name=f"ctx{i}")
                    for i in range(NKP)
                ]

                for pair in range(H // 2):
                    h0 = pair * 2
                    qtile = QT[pair]
                    ktile = KT[pair]
                    # both heads' score matmuls first, so the ACT exp
                    # pipeline runs ahead of the PV accumulation chain
                    pts = {}
                    for sub in range(2):
                        hh = h0 + sub
                        off = 64 * sub
                        pt2 = [
                            p_pt8.tile([PD, 2, T], FP8, tag="pt8",
                                       name=f"pt{hh}_{kp}")
                            for kp in range(NTC // 2)
                        ]
                        for kb in range(NTC):
                            sp = ps_a.tile([PD, T], F32, tag="pa",
                                           name=f"sp{hh}_{kb}")
                            nc.tensor.matmul(
                                sp[:],
                                ktile[off : off + 64, kb * PD : (kb + 1) * PD],
                                qtile[off : off + 64, :],
                                start=True,
                                stop=True,
                            )
                            nc.scalar.activation(
                                pt2[kb // 2][:, kb % 2, :], sp[:], AF.Exp,
                                bias=(mask_t[:, kb : kb + 1] if use_mask else 0.0),
                                scale=SCALE,
                            )
                        pts[sub] = pt2
                    den2 = p_den.tile([2, T], F32R, tag="den", name=f"den{pair}")
                    cps = []
                    for sub in range(2):
                        hh = h0 + sub
                        cp = ps_c.tile([PD, T], F32, tag="ctxp", name=f"cp{hh}")
                        for kp in range(NTC // 2):
                            nc.tensor.matmul(
                                cp[:],
                                V8[kp][:, :, PD * hh : PD * hh + PD],
                                pts[sub][kp][:],
                                start=(kp == 0),
                                stop=(kp == NTC // 2 - 1),
                                perf_mode=DR,
                            )
                        cps.append((hh, cp))
                    # each cp holds its head's denominator in one of rows
                    # 64:66 (zeros in the other); their sum is [den0; den1]
                    d0 = p_den.tile([2, T], F32R, tag="d0", name=f"d0{pair}")
                    nc.scalar.copy(d0[:], cps[0][1][64:66, :])
                    nc.vector.tensor_tensor(
                        den2[:], cps[1][1][64:66, :], d0[:], op=OP.add
                    )
                    rden = p_den.tile([2, T], F32R, tag="rden", name=f"rd{pair}")
                    with nc.allow_low_precision(reason="f32r is fp32-width"):
                        nc.vector.reciprocal(rden[:], den2[:])
                    # broadcast 64/den across the pair's 128 rows (K=2 outer
                    # product; sel2 rows are 64*indicator vectors)
                    pr = ps_b.tile([PD, T], F32, tag="pb", name=f"pr{pair}")
                    nc.tensor.matmul(
                        pr[:], sel2_t[:], rden[:], start=True, stop=True
                    )
                    rsb = p_den.tile([PD, T], F32R, tag="rsb", name=f"rs{pair}")
                    nc.scalar.copy(rsb[:], pr[:])
                    for hh, cp in cps:
                        off = 64 * (hh % 2)
                        dst = ctxT8[hh // 4][
                            off : off + 64, (hh // 2) % 2, :
                        ]
                        nc.vector.tensor_tensor(
                            dst, cp[0:64, :], rsb[off : off + 64, :], op=OP.mult
                        )

                # ---- output block: z = x @ W + residual, then LN ----
                def out_block(mm, descale, res_tiles, badd_d,
                              use_badd, lnw_d_, lnb_d_, use_ln, is_last):
                    outs = []
                    if use_badd:
                        badd_t = p_bc.tile([PD, D], F32, tag="badd")
                        nc.sync.dma_start(badd_t[:], badd_d.ap()[l])
                    if use_ln:
                        lnw_t = p_bc.tile([PD, D], F32, tag="lnw")
                        nc.sync.dma_start(lnw_t[:], lnw_d_.ap()[l])
                        lnb_t = p_bc.tile([PD, D], F32, tag="lnb")
                        nc.sync.dma_start(lnb_t[:], lnb_d_.ap()[l])
                    for tc_i in range(NTC):
                        z = p_z.tile([PD, D], F32, tag="z")
                        s01 = p_sm.tile([PD, NG], F32, tag="s01")
                        for ng in range(NG):
                            pp = ps_b.tile([PD, GW], F32, tag="pb")
                            mm(pp, tc_i, slice(ng * GW, (ng + 1) * GW))
                            sl = slice(ng * GW, (ng + 1) * GW)
                            if use_badd:
                                nc.vector.scalar_tensor_tensor(
                                    z[:, sl], pp[:], descale, res_tiles[tc_i][:, sl],
                                    op0=OP.mult, op1=OP.add,
                                )
                                nc.vector.scalar_tensor_tensor(
                                    z[:, sl], z[:, sl], 1.0, badd_t[:, sl],
                                    op0=OP.mult, op1=OP.add,
                                    accum_out=s01[:, ng : ng + 1],
                                )
                            else:
                                nc.vector.scalar_tensor_tensor(
                                    z[:, sl], pp[:], descale, res_tiles[tc_i][:, sl],
                                    op0=OP.mult, op1=OP.add,
                                    accum_out=s01[:, ng : ng + 1],
                                )
                        # layernorm over the full 768-wide row; small
                        # SBUF-only scalar ops ride the idle Pool engine
                        ssum = p_sm.tile([PD, 1], F32, tag="ssum")
                        nc.gpsimd.tensor_tensor(
                            ssum[:], s01[:, 0:1], s01[:, 1:2], op=OP.add
                        )
                        uneg = p_sm.tile([PD, 1], F32, tag="uneg")
                        nc.gpsimd.tensor_scalar_mul(uneg[:], ssum[:], -1.0 / D)
                        sq = p_z.tile([PD, D], F32, tag="sq")
                        ssq = p_sm.tile([PD, 1], F32, tag="ssq")
                        nc.scalar.activation(
                            sq[:], z[:], AF.Square, bias=uneg[:], scale=1.0,
                            accum_out=ssq[:],
                        )
                        var_eps = p_sm.tile([PD, 1], F32, tag="vareps")
                        nc.gpsimd.tensor_scalar(
                            var_eps[:], ssq[:], 1.0 / D, EPS, op0=OP.mult, op1=OP.add
                        )
                        stdev = p_sm.tile([PD, 1], F32, tag="stdev")
                        nc.scalar.sqrt(stdev[:], var_eps[:])
                        rstd = p_sm.tile([PD, 1], F32, tag="rstd")
                        nc.vector.reciprocal(rstd[:], stdev[:])
                        urneg = p_sm.tile([PD, 1], F32, tag="urneg")
                        nc.gpsimd.tensor_tensor(
                            urneg[:], uneg[:], rstd[:], op=OP.mult
                        )
                        o = p_hid.tile([PD, D], F32R, tag="hid")
                        if use_ln:
                            on = p_z.tile([PD, D], F32, tag="sq")
                            nc.gpsimd.tensor_scalar(
                                on[:], z[:], rstd[:], urneg[:], op0=OP.mult, op1=OP.add
                            )
                            nc.gpsimd.tensor_tensor(
                                on[:], on[:], lnw_t[:], op=OP.mult
                            )
                            nc.gpsimd.tensor_tensor(
                                o[:], on[:], lnb_t[:], op=OP.add
                            )
                        else:
                            nc.gpsimd.tensor_scalar(
                                o[:], z[:], rstd[:], urneg[:], op0=OP.mult, op1=OP.add
                            )
                        if is_last:
                            nc.sync.dma_start(
                                out_d.ap()[tc_i * PD : (tc_i + 1) * PD, :], o[:]
                            )
                        outs.append(o)
                    return outs

                def mm_out1(pp, tc_i, sl):
                    for i in range(NKP):
                        nc.tensor.matmul(
                            pp[:],
                            ctxT8[i][:, :, tc_i * PD : (tc_i + 1) * PD],
                            w8ap(wo1_t, i, sl),
                            start=(i == 0),
                            stop=(i == NKP - 1),
                            perf_mode=DR,
                        )

                a_tiles = out_block(
                    mm_out1, 1.0 / (WS * WS), h_tiles, b1_d, use_b1,
                    ln1w_d, ln1b_d, use_ln1, False,
                )
                # out2's update (a @ Wo2) is ~0.55x the residual scale, so
                # fp8 noise there dominates the whole kernel's error; run
                # this one GEMM in bf16 instead.
                aTb = transpose_to(a_tiles, p_ht8, "atb", BF16, False)

                def mm_out2(pp, tc_i, sl):
                    for kc in range(NKC):
                        nc.tensor.matmul(
                            pp[:],
                            aTb[kc][:, tc_i * PD : (tc_i + 1) * PD],
                            wo2_t[:, kc * D + sl.start : kc * D + sl.stop],
                            start=(kc == 0),
                            stop=(kc == NKC - 1),
                        )

                h_tiles = out_block(
                    mm_out2, 1.0, a_tiles, b2_d, use_b2,
                    ln2w_d, ln2b_d, use_ln2, l == L - 1,
                )

    if split_waits:
        import bass_rust

        _split_excess_waits(nc, mybir, bass_rust)
    return nc


def _fp8np():
    from concourse import mybir

    return mybir.dt.np(mybir.dt.float8e4)


def prep_inputs(inputs):
    """Host-side folds. Returns (flags, per-core list)."""
    g = {k: np.asarray(v, dtype=np.float32) for k, v in inputs.items()}
    fp8 = _fp8np()

    b1 = np.einsum("ld,ldo->lo", g["bv"], g["Wo1"]) + g["bo1"]
    b2 = g["bo2"]

    flags = {
        "use_mask": bool(np.any(g["attention_mask"])),
        "use_bq": bool(np.any(g["bq"])),
        "use_bk": bool(np.any(g["bk"])),
        "use_b1": bool(np.any(b1)),
        "use_b2": bool(np.any(b2)),
        "use_ln1": bool(np.any(g["ln1_w"] != 1.0) or np.any(g["ln1_b"])),
        "use_ln2": bool(np.any(g["ln2_w"] != 1.0) or np.any(g["ln2_b"])),
    }

    def wfmt_bf(w):
        import ml_dtypes
        r = w.reshape(L, NKC, PD, D).transpose(0, 2, 1, 3).reshape(L, PD, NKC * D)
        return np.ascontiguousarray(r.astype(ml_dtypes.bfloat16))

    def wfmt8(w):
        # [L,768,768] -> [L, 128, (pair=3, two=2, dout=768)] fp8 of 64*w
        r = w.reshape(L, NKP, 2, PD, D).transpose(0, 3, 1, 2, 4).reshape(
            L, PD, NKC * D
        )
        return np.ascontiguousarray(
            np.clip(r * WS, -FP8_MAX, FP8_MAX).astype(fp8)
        )

    def bfmt(b):
        return np.ascontiguousarray(
            b.reshape(L, NKC, PD).transpose(2, 0, 1).reshape(PD, L * NKC)
        )

    shared = {
        "wq": wfmt8(g["Wq"]),
        "wk": wfmt8(g["Wk"]),
        "wv": wfmt8(g["Wv"]),
        "wo1": wfmt8(g["Wo1"]),
        "wo2": wfmt_bf(g["Wo2"]),
        "iden": np.eye(PD, dtype=np.float32),
    }
    if flags["use_bq"]:
        shared["bq"] = bfmt(g["bq"])
    if flags["use_bk"]:
        shared["bk"] = bfmt(g["bk"])
    sel2 = np.zeros((2, PD), dtype=np.float32)
    sel2[0, :64] = WS
    sel2[1, 64:] = WS
    shared["sel2"] = sel2
    if flags["use_b1"]:
        shared["b1bc"] = np.ascontiguousarray(
            np.broadcast_to(b1[:, None, :], (L, PD, D))
        )
    if flags["use_b2"]:
        shared["b2bc"] = np.ascontiguousarray(
            np.broadcast_to(b2[:, None, :], (L, PD, D))
        )
    if flags["use_ln1"]:
        shared["ln1wbc"] = np.ascontiguousarray(
            np.broadcast_to(g["ln1_w"][:, None, :], (L, PD, D))
        )
        shared["ln1bbc"] = np.ascontiguousarray(
            np.broadcast_to(g["ln1_b"][:, None, :], (L, PD, D))
        )
    if flags["use_ln2"]:
        shared["ln2wbc"] = np.ascontiguousarray(
            np.broadcast_to(g["ln2_w"][:, None, :], (L, PD, D))
        )
        shared["ln2bbc"] = np.ascontiguousarray(
            np.broadcast_to(g["ln2_b"][:, None, :], (L, PD, D))
        )

    per_core = []
    for b in range(B):
        m = dict(shared)
        m["qs"] = np.ascontiguousarray(g["query_states"][b])
        m["hs"] = np.ascontiguousarray(g["hidden_states"][b])
        if flags["use_mask"]:
            m["mask"] = np.ascontiguousarray(
                g["attention_mask"][b].reshape(NTC, PD).T
            )
        per_core.append(m)
    return flags, per_core


TRACE = False
LAST_EXEC_NS = None
LAST_RESULTS = None


def kernel(**inputs):
    global LAST_EXEC_NS, LAST_RESULTS
    from concourse.bass_utils import run_bass_kernel_spmd

    flags, per_core = prep_inputs(inputs)
    nc = build_nc(flags)
    kw = {}
    if TRACE:
        kw = dict(trace=True, tmpdir="/root/problem/trace_out")
        import os

        os.makedirs("/root/problem/trace_out", exist_ok=True)
    res = run_bass_kernel_spmd(nc, per_core, core_ids=list(range(B)), **kw)
    LAST_EXEC_NS = res.exec_time_ns
    LAST_RESULTS = res
    out = np.stack([np.asarray(res.results[b]["out"]) for b in range(B)], axis=0)
    return out.astype(np.float32)


# revision 12
# speedup vs baseline: 1.2466x; 1.0950x over previous
"""BERT encoder (12 layers, B=8 T=512 D=768 H=12) on 8 Trainium2 NeuronCores.

Strategy: pure data parallelism — core b runs the full 12-layer stack for
batch element b. No collectives.

Tensor-engine budget is the bottleneck, and matmul cost on TRN2 is
(output free size) x (cycles/row) independent of K and M, so the kernel
maximizes contraction per instruction: all five per-layer GEMMs and the
PV product run in fp8e4m3 with DoubleRow perf mode (two 128-deep k-tiles
per instruction, 0.5 cycles/row -> 4x fewer PE column-cycles than f32r).
QK^T scores stay float32r (K=64 can't exploit DoubleRow without a costly
re-fold). Softmax: ACT Exp with the 1/sqrt(dh) scale and mask folded in,
emitting P^T directly in fp8; the denominator comes out of the PV matmul
via an appended ones-column in V, is reciprocal'd on a [2,T] tile and
broadcast to 128 partitions by a single K=2 PE outer product.

fp8 scaling (all powers of two, exact): weights x64; Q/K/V descaled by
1/64 at PSUM evac; ctx scaled x64 by the denominator broadcast; the
output GEMM descales 1/4096 (ctx x wo1) or 1/64 (a x wo2) in the fused
residual-add. Activations (std ~1) cast straight to fp8.

Engine placement: exp/sqrt/K-evac on ACT; residual stt, divides, LN
finals, Q-evac on DVE; transpose evacs, V evac, denominator copies, LN
variance (sum z^2 via stt-accum) on the otherwise-idle Pool engine.

Host-side folds (exact, negligible FLOPs):
  - V bias folded through Wo1: b1 = bv @ Wo1 + bo1 (rows of softmax sum to 1)
  - weights pre-reshaped to the DoubleRow lhsT chunk layout
Zero biases / zero mask / identity LN affine (which is what
reference.setup_inputs() produces) skip their device ops entirely, but the
general paths are implemented and selected when inputs are nonzero.
"""

import numpy as np

L, B, T, D, H, DH = 12, 8, 512, 768, 12, 64
PD = 128
NKC = D // PD   # 6 contraction chunks of 128
NKP = NKC // 2  # 3 DoubleRow chunk-pairs of 256
NTC = T // PD   # 4 token chunks
NG = 2          # N-groups per 768-wide output (384 each)
GW = D // NG    # 384
EPS = 1e-12
SCALE = 1.0 / np.sqrt(np.float32(DH))
WS = 64.0       # fp8 weight scale (power of two, exact)
FP8_MAX = 240.0


def _split_excess_waits(nc, mybir, bass_rust, max_waits=1):
    """walrus codegen rejects instructions carrying more than a couple of
    sync waits; hoist excess waits onto same-engine NoOps placed before."""
    n = 0
    for f in nc.m.functions:
        for bb in f.blocks:
            new_insts = []
            changed = False
            for inst in bb.instructions:
                si = inst.sync_info
                if si is not None and len(si.on_wait) > max_waits:
                    waits = list(si.on_wait)
                    excess = waits[: len(waits) - max_waits]
                    for i in range(0, len(excess), max_waits):
                        chunk = excess[i : i + max_waits]
                        n += 1
                        nop = mybir.InstNoOp(
                            name=f"I-waitsplit-{n}", ins=[], outs=[]
                        )
                        nop.engine = inst.engine
                        nop.sync_info = bass_rust.SyncInfo(
                            on_wait=chunk, on_update=[]
                        )
                        new_insts.append(nop)
                        changed = True
                    si.on_wait = waits[len(waits) - max_waits :]
                new_insts.append(inst)
            if changed:
                bb.instructions[:] = new_insts
    return n


def build_nc(flags, split_waits=True):
    """Build the per-core Bass module. flags: dict of general-path toggles."""
    import concourse.bass as bass
    import concourse.tile as tile
    from concourse import mybir

    F32 = mybir.dt.float32
    F32R = mybir.dt.float32r
    FP8 = mybir.dt.float8e4
    AF = mybir.ActivationFunctionType
    OP = mybir.AluOpType
    DR = mybir.MatmulPerfMode.DoubleRow

    use_mask = flags["use_mask"]
    use_bq = flags["use_bq"]
    use_bk = flags["use_bk"]
    use_b1 = flags["use_b1"]
    use_b2 = flags["use_b2"]
    use_ln1 = flags["use_ln1"]
    use_ln2 = flags["use_ln2"]

    nc = bass.Bass("TRN2", target_bir_lowering=False, debug=False)

    qs_d = nc.dram_tensor("qs", [T, D], F32R, kind="ExternalInput")
    hs_d = nc.dram_tensor("hs", [T, D], F32R, kind="ExternalInput")
    BF16 = mybir.dt.bfloat16
    w_d = {
        name: nc.dram_tensor(name, [L, PD, NKC * D], FP8, kind="ExternalInput")
        for name in ("wq", "wk", "wv", "wo1")
    }
    w_d["wo2"] = nc.dram_tensor("wo2", [L, PD, NKC * D], BF16, kind="ExternalInput")
    iden_d = nc.dram_tensor("iden", [PD, PD], F32R, kind="ExternalInput")
    bq_d = nc.dram_tensor("bq", [PD, L * NKC], F32, kind="ExternalInput") if use_bq else None
    bk_d = nc.dram_tensor("bk", [PD, L * NKC], F32, kind="ExternalInput") if use_bk else None
    mask_d = nc.dram_tensor("mask", [PD, NTC], F32, kind="ExternalInput") if use_mask else None
    sel2_d = nc.dram_tensor("sel2", [2, PD], F32R, kind="ExternalInput")
    b1_d = nc.dram_tensor("b1bc", [L, PD, D], F32, kind="ExternalInput") if use_b1 else None
    b2_d = nc.dram_tensor("b2bc", [L, PD, D], F32, kind="ExternalInput") if use_b2 else None
    ln1w_d = nc.dram_tensor("ln1wbc", [L, PD, D], F32, kind="ExternalInput") if use_ln1 else None
    ln1b_d = nc.dram_tensor("ln1bbc", [L, PD, D], F32, kind="ExternalInput") if use_ln1 else None
    ln2w_d = nc.dram_tensor("ln2wbc", [L, PD, D], F32, kind="ExternalInput") if use_ln2 else None
    ln2b_d = nc.dram_tensor("ln2bbc", [L, PD, D], F32, kind="ExternalInput") if use_ln2 else None
    out_d = nc.dram_tensor("out", [T, D], F32R, kind="ExternalOutput")

    def w8ap(t, i, sl):
        """[128, NKC*D] fp8 weight tile -> [128, 2, sl] DoubleRow slice for
        chunk-pair i."""
        return t[:].rearrange("p (i two d) -> p i two d", two=2, d=D)[:, i, :, sl]

    with tile.TileContext(nc) as tc:
        import contextlib

        with contextlib.ExitStack() as ctx:
            p_w = ctx.enter_context(tc.tile_pool(name="w", bufs=4))
            p_qt8 = ctx.enter_context(tc.tile_pool(name="qt8", bufs=3))
            p_hid = ctx.enter_context(tc.tile_pool(name="hid", bufs=8))
            p_ht8 = ctx.enter_context(tc.tile_pool(name="ht8", bufs=7))
            p_qk = ctx.enter_context(tc.tile_pool(name="qk", bufs=14))
            p_ctx8 = ctx.enter_context(tc.tile_pool(name="ctx8", bufs=4))
            p_v8 = ctx.enter_context(tc.tile_pool(name="v8", bufs=3))
            p_pt8 = ctx.enter_context(tc.tile_pool(name="pt8", bufs=9))
            p_den = ctx.enter_context(tc.tile_pool(name="den", bufs=4))
            p_z = ctx.enter_context(tc.tile_pool(name="z", bufs=3))
            p_sm = ctx.enter_context(tc.tile_pool(name="sm", bufs=2))
            p_c1 = ctx.enter_context(tc.tile_pool(name="c1", bufs=1))
            p_bc = ctx.enter_context(tc.tile_pool(name="bc", bufs=2))
            ps_a = ctx.enter_context(tc.tile_pool(name="psA", bufs=4, space="PSUM"))
            ps_b = ctx.enter_context(tc.tile_pool(name="psB", bufs=2, space="PSUM"))
            ps_c = ctx.enter_context(tc.tile_pool(name="psC", bufs=2, space="PSUM"))

            # ---- one-time constants / inputs ----
            iden = p_c1.tile([PD, PD], F32R, tag="iden")
            nc.sync.dma_start(iden[:], iden_d.ap())
            if use_bq:
                bq_t = p_c1.tile([PD, L * NKC], F32, tag="bq")
                nc.sync.dma_start(bq_t[:], bq_d.ap())
            if use_bk:
                bk_t = p_c1.tile([PD, L * NKC], F32, tag="bk")
                nc.sync.dma_start(bk_t[:], bk_d.ap())
            if use_mask:
                mask_t = p_c1.tile([PD, NTC], F32, tag="mask")
                nc.sync.dma_start(mask_t[:], mask_d.ap())
            sel2_t = p_c1.tile([2, PD], F32R, tag="sel2")
            nc.sync.dma_start(sel2_t[:], sel2_d.ap())

            qs_n = []
            for tc_i in range(NTC):
                t = p_hid.tile([PD, D], F32R, tag="hid")
                nc.sync.dma_start(t[:], qs_d.ap()[tc_i * PD : (tc_i + 1) * PD, :])
                qs_n.append(t)
            h_tiles = []
            for tc_i in range(NTC):
                t = p_hid.tile([PD, D], F32R, tag="hid")
                nc.sync.dma_start(t[:], hs_d.ap()[tc_i * PD : (tc_i + 1) * PD, :])
                h_tiles.append(t)

            def transpose_to(src_tiles, pool, tag, dt, pair):
                """[T, D] (4x[128,768] f32r) -> [D, T], transposed on PE
                (f32r) and cast at evac on Pool. pair=True packs DoubleRow
                chunk-pairs (NKP tiles of [128, 2, T]); else NKC flat tiles
                of [128, T]."""
                out = []
                for i in range(NKP if pair else NKC):
                    dst = pool.tile([PD, 2, T] if pair else [PD, T], dt, tag=tag)
                    for j in range(2 if pair else 1):
                        kc = 2 * i + j if pair else i
                        pt = ps_b.tile([PD, T], F32R, tag="pb")
                        for tc_i in range(NTC):
                            nc.tensor.transpose(
                                pt[:, tc_i * PD : (tc_i + 1) * PD],
                                src_tiles[tc_i][:, kc * PD : (kc + 1) * PD],
                                iden[:],
                            )
                        nc.vector.tensor_copy(dst[:, j, :] if pair else dst[:], pt[:])
                    out.append(dst)
                return out

            qT8 = transpose_to(qs_n, p_qt8, "qt8", FP8, True)

            # ---- layers ----
            for l in range(L):
                wq_t = p_w.tile([PD, NKC * D], FP8, tag="w")
                nc.sync.dma_start(wq_t[:], w_d["wq"].ap()[l])
                wk_t = p_w.tile([PD, NKC * D], FP8, tag="w")
                nc.sync.dma_start(wk_t[:], w_d["wk"].ap()[l])
                wv_t = p_w.tile([PD, NKC * D], FP8, tag="w")
                nc.sync.dma_start(wv_t[:], w_d["wv"].ap()[l])

                hT8 = transpose_to(h_tiles, p_ht8, "ht8", FP8, True)

                # Q^T, K^T: [D, T] f32r, d_out on partitions. DoubleRow over
                # the 768-deep contraction (3 chained K=256 matmuls).
                def proj_T(w_tile, rhs8, bias_t, use_bias, on_act):
                    outs = []
                    for mc in range(NKC):
                        pp = ps_a.tile([PD, T], F32, tag="pa")
                        sl = slice(mc * PD, (mc + 1) * PD)
                        for i in range(NKP):
                            nc.tensor.matmul(
                                pp[:],
                                w8ap(w_tile, i, sl),
                                rhs8[i][:],
                                start=(i == 0),
                                stop=(i == NKP - 1),
                                perf_mode=DR,
                            )
                        dst = p_qk.tile([PD, T], F32R, tag="qk")
                        bias_ap = (
                            bias_t[:, l * NKC + mc : l * NKC + mc + 1]
                            if use_bias
                            else 0.0
                        )
                        if on_act:
                            nc.scalar.activation(
                                dst[:], pp[:], AF.Identity,
                                bias=bias_ap, scale=1.0 / WS,
                            )
                        else:
                            nc.vector.tensor_scalar(
                                dst[:], pp[:], 1.0 / WS, bias_ap,
                                op0=OP.mult, op1=OP.add,
                            )
                        outs.append(dst)
                    return outs

                QT = proj_T(wq_t, qT8, bq_t if use_bq else None, use_bq, False)
                KT = proj_T(wk_t, hT8, bk_t if use_bk else None, use_bk, True)

                # V: fp8, augmented layout per kb-pair: [128, 2, H*128];
                # head h at cols 128h..128h+63; a ones column at 128h+64
                # (h even) or 128h+65 (h odd), zeros elsewhere, so the PV
                # product of a head pair emits the two softmax denominators
                # in the partition-aligned rows 64:66. The pad to a 128
                # stride keeps dual-fp8 LDWEIGHTS legal (M must be 128) and
                # costs nothing: matmul time only scales with N.
                V8 = []
                for tp in range(NTC // 2):
                    vt = p_v8.tile([PD, 2, H * PD], FP8, tag="v8")
                    for j in range(2):
                        tc_i = 2 * tp + j
                        pairs = vt[:, j, :].rearrange("p (m c) -> p m c", c=256)
                        nc.gpsimd.memset(pairs[:, :, 64:128], 0.0)
                        nc.gpsimd.memset(pairs[:, :, 192:256], 0.0)
                        nc.gpsimd.memset(pairs[:, :, 64:65], 1.0)
                        nc.gpsimd.memset(pairs[:, :, 193:194], 1.0)
                        for ng in range(NG):
                            pp = ps_b.tile([PD, GW], F32, tag="pb")
                            sl = slice(ng * GW, (ng + 1) * GW)
                            for i in range(NKP):
                                nc.tensor.matmul(
                                    pp[:],
                                    hT8[i][:, :, tc_i * PD : (tc_i + 1) * PD],
                                    w8ap(wv_t, i, sl),
                                    start=(i == 0),
                                    stop=(i == NKP - 1),
                                    perf_mode=DR,
                                )
                            dst = vt[:, j, ng * 6 * PD : (ng + 1) * 6 * PD].rearrange(
                                "p (h c) -> p h c", c=PD
                            )[:, :, 0:64]
                            src_ = pp[:].rearrange("p (h c) -> p h c", c=64)
                            nc.vector.tensor_scalar_mul(dst, src_, 1.0 / WS)
                    V8.append(vt)

                wo1_t = p_w.tile([PD, NKC * D], FP8, tag="w")
                nc.sync.dma_start(wo1_t[:], w_d["wo1"].ap()[l])
                wo2_t = p_w.tile([PD, NKC * D], BF16, tag="wbf")
                nc.sync.dma_start(wo2_t[:], w_d["wo2"].ap()[l])

                # ctx^T in fp8 DoubleRow pair layout: NKP tiles [128, 2, T].
                # head h lives in tile h//4, sub (h//2)%2, rows 64*(h%2).
                ctxT8 = [
                    p_ctx8.tile([PD, 2, T], FP8, tag="ctx8", name=f"ctx{i}")
                    for i in range(NKP)
                ]

                for pair in range(H // 2):
                    h0 = pair * 2
                    qtile = QT[pair]
                    ktile = KT[pair]
                    # both heads' score matmuls first, so the ACT exp
                    # pipeline runs ahead of the PV accumulation chain
                    pts = {}
                    for sub in range(2):
                        hh = h0 + sub
                        off = 64 * sub
                        pt2 = [
                            p_pt8.tile([PD, 2, T], FP8, tag="pt8",
                                       name=f"pt{hh}_{kp}")
                            for kp in range(NTC // 2)
                        ]
                        for kb in range(NTC):
                            sp = ps_a.tile([PD, T], F32, tag="pa",
                                           name=f"sp{hh}_{kb}")
                            nc.tensor.matmul(
                                sp[:],
                                ktile[off : off + 64, kb * PD : (kb + 1) * PD],
                                qtile[off : off + 64, :],
                                start=True,
                                stop=True,
                            )
                            nc.scalar.activation(
                                pt2[kb // 2][:, kb % 2, :], sp[:], AF.Exp,
                                bias=(mask_t[:, kb : kb + 1] if use_mask else 0.0),
                                scale=SCALE,
                            )
                        pts[sub] = pt2
                    den2 = p_den.tile([2, T], F32R, tag="den", name=f"den{pair}")
                    cps = []
                    for sub in range(2):
                        hh = h0 + sub
                        cp = ps_c.tile([PD, T], F32, tag="ctxp", name=f"cp{hh}")
                        for kp in range(NTC // 2):
                            nc.tensor.matmul(
                                cp[:],
                                V8[kp][:, :, PD * hh : PD * hh + PD],
                                pts[sub][kp][:],
                                start=(kp == 0),
                                stop=(kp == NTC // 2 - 1),
                                perf_mode=DR,
                            )
                        cps.append((hh, cp))
                    # each cp holds its head's denominator in one of rows
                    # 64:66 (zeros in the other); their sum is [den0; den1]
                    d0 = p_den.tile([2, T], F32R, tag="d0", name=f"d0{pair}")
                    nc.scalar.copy(d0[:], cps[0][1][64:66, :])
                    nc.vector.tensor_tensor(
                        den2[:], cps[1][1][64:66, :], d0[:], op=OP.add
                    )
                    # 1/den as exp(-ln(den)) on ACT: DVE reciprocal costs
                    # ~6.6 ns/column regardless of partition count, far more
                    # than two ACT table ops
                    lden = p_den.tile([2, T], F32R, tag="lden", name=f"ld{pair}")
                    nc.scalar.activation(lden[:], den2[:], AF.Ln, bias=0.0, scale=1.0)
                    rden = p_den.tile([2, T], F32R, tag="rden", name=f"rd{pair}")
                    nc.scalar.activation(rden[:], lden[:], AF.Exp, bias=0.0, scale=-1.0)
                    # broadcast 64/den across the pair's 128 rows (K=2 outer
                    # product; sel2 rows are 64*indicator vectors)
                    pr = ps_b.tile([PD, T], F32, tag="pb", name=f"pr{pair}")
                    nc.tensor.matmul(
                        pr[:], sel2_t[:], rden[:], start=True, stop=True
                    )
                    rsb = p_den.tile([PD, T], F32R, tag="rsb", name=f"rs{pair}")
                    nc.vector.tensor_copy(rsb[:], pr[:])
                    for hh, cp in cps:
                        off = 64 * (hh % 2)
                        dst = ctxT8[hh // 4][
                            off : off + 64, (hh // 2) % 2, :
                        ]
                        nc.vector.tensor_tensor(
                            dst, cp[0:64, :], rsb[off : off + 64, :], op=OP.mult
                        )

                # ---- output block: z = x @ W + residual, then LN ----
                def out_block(mm, descale, res_tiles, badd_d,
                              use_badd, lnw_d_, lnb_d_, use_ln, is_last):
                    outs = []
                    if use_badd:
                        badd_t = p_bc.tile([PD, D], F32, tag="badd")
                        nc.sync.dma_start(badd_t[:], badd_d.ap()[l])
                    if use_ln:
                        lnw_t = p_bc.tile([PD, D], F32, tag="lnw")
                        nc.sync.dma_start(lnw_t[:], lnw_d_.ap()[l])
                        lnb_t = p_bc.tile([PD, D], F32, tag="lnb")
                        nc.sync.dma_start(lnb_t[:], lnb_d_.ap()[l])
                    for tc_i in range(NTC):
                        z = p_z.tile([PD, D], F32, tag="z")
                        s01 = p_sm.tile([PD, NG], F32, tag="s01")
                        for ng in range(NG):
                            pp = ps_b.tile([PD, GW], F32, tag="pb")
                            mm(pp, tc_i, slice(ng * GW, (ng + 1) * GW))
                            sl = slice(ng * GW, (ng + 1) * GW)
                            if use_badd:
                                nc.vector.scalar_tensor_tensor(
                                    z[:, sl], pp[:], descale, res_tiles[tc_i][:, sl],
                                    op0=OP.mult, op1=OP.add,
                                )
                                nc.vector.scalar_tensor_tensor(
                                    z[:, sl], z[:, sl], 1.0, badd_t[:, sl],
                                    op0=OP.mult, op1=OP.add,
                                    accum_out=s01[:, ng : ng + 1],
                                )
                            else:
                                nc.vector.scalar_tensor_tensor(
                                    z[:, sl], pp[:], descale, res_tiles[tc_i][:, sl],
                                    op0=OP.mult, op1=OP.add,
                                    accum_out=s01[:, ng : ng + 1],
                                )
                        # layernorm over the full 768-wide row; small
                        # SBUF-only scalar ops ride the idle Pool engine
                        ssum = p_sm.tile([PD, 1], F32, tag="ssum")
                        nc.gpsimd.tensor_tensor(
                            ssum[:], s01[:, 0:1], s01[:, 1:2], op=OP.add
                        )
                        uneg = p_sm.tile([PD, 1], F32, tag="uneg")
                        nc.gpsimd.tensor_scalar_mul(uneg[:], ssum[:], -1.0 / D)
                        sq = p_z.tile([PD, D], F32, tag="sq")
                        ssq = p_sm.tile([PD, 1], F32, tag="ssq")
                        nc.scalar.activation(
                            sq[:], z[:], AF.Square, bias=uneg[:], scale=1.0,
                            accum_out=ssq[:],
                        )
                        var_eps = p_sm.tile([PD, 1], F32, tag="vareps")
                        nc.gpsimd.tensor_scalar(
                            var_eps[:], ssq[:], 1.0 / D, EPS, op0=OP.mult, op1=OP.add
                        )
                        stdev = p_sm.tile([PD, 1], F32, tag="stdev")
                        nc.scalar.sqrt(stdev[:], var_eps[:])
                        rstd = p_sm.tile([PD, 1], F32, tag="rstd")
                        nc.vector.reciprocal(rstd[:], stdev[:])
                        urneg = p_sm.tile([PD, 1], F32, tag="urneg")
                        nc.gpsimd.tensor_tensor(
                            urneg[:], uneg[:], rstd[:], op=OP.mult
                        )
                        o = p_hid.tile([PD, D], F32R, tag="hid")
                        if use_ln:
                            on = p_z.tile([PD, D], F32, tag="sq")
                            nc.gpsimd.tensor_scalar(
                                on[:], z[:], rstd[:], urneg[:], op0=OP.mult, op1=OP.add
                            )
                            nc.gpsimd.tensor_tensor(
                                on[:], on[:], lnw_t[:], op=OP.mult
                            )
                            nc.gpsimd.tensor_tensor(
                                o[:], on[:], lnb_t[:], op=OP.add
                            )
                        else:
                            nc.gpsimd.tensor_scalar(
                                o[:], z[:], rstd[:], urneg[:], op0=OP.mult, op1=OP.add
                            )
                        if is_last:
                            nc.sync.dma_start(
                                out_d.ap()[tc_i * PD : (tc_i + 1) * PD, :], o[:]
                            )
                        outs.append(o)
                    return outs

                def mm_out1(pp, tc_i, sl):
                    for i in range(NKP):
                        nc.tensor.matmul(
                            pp[:],
                            ctxT8[i][:, :, tc_i * PD : (tc_i + 1) * PD],
                            w8ap(wo1_t, i, sl),
                            start=(i == 0),
                            stop=(i == NKP - 1),
                            perf_mode=DR,
                        )

                a_tiles = out_block(
                    mm_out1, 1.0 / (WS * WS), h_tiles, b1_d, use_b1,
                    ln1w_d, ln1b_d, use_ln1, False,
                )
                # out2's update (a @ Wo2) is ~0.55x the residual scale, so
                # fp8 noise there dominates the whole kernel's error; run
                # this one GEMM in bf16 instead.
                aTb = transpose_to(a_tiles, p_ht8, "atb", BF16, False)

                def mm_out2(pp, tc_i, sl):
                    for kc in range(NKC):
                        nc.tensor.matmul(
                            pp[:],
                            aTb[kc][:, tc_i * PD : (tc_i + 1) * PD],
                            wo2_t[:, kc * D + sl.start : kc * D + sl.stop],
                            start=(kc == 0),
                            stop=(kc == NKC - 1),
                        )

                h_tiles = out_block(
                    mm_out2, 1.0, a_tiles, b2_d, use_b2,
                    ln2w_d, ln2b_d, use_ln2, l == L - 1,
                )

    if split_waits:
        import bass_rust

        _split_excess_waits(nc, mybir, bass_rust)
    return nc


def _fp8np():
    from concourse import mybir

    return mybir.dt.np(mybir.dt.float8e4)


def prep_inputs(inputs):
    """Host-side folds. Returns (flags, per-core list)."""
    g = {k: np.asarray(v, dtype=np.float32) for k, v in inputs.items()}
    fp8 = _fp8np()

    b1 = np.einsum("ld,ldo->lo", g["bv"], g["Wo1"]) + g["bo1"]
    b2 = g["bo2"]

    flags = {
        "use_mask": bool(np.any(g["attention_mask"])),
        "use_bq": bool(np.any(g["bq"])),
        "use_bk": bool(np.any(g["bk"])),
        "use_b1": bool(np.any(b1)),
        "use_b2": bool(np.any(b2)),
        "use_ln1": bool(np.any(g["ln1_w"] != 1.0) or np.any(g["ln1_b"])),
        "use_ln2": bool(np.any(g["ln2_w"] != 1.0) or np.any(g["ln2_b"])),
    }

    def wfmt_bf(w):
        import ml_dtypes
        r = w.reshape(L, NKC, PD, D).transpose(0, 2, 1, 3).reshape(L, PD, NKC * D)
        return np.ascontiguousarray(r.astype(ml_dtypes.bfloat16))

    def wfmt8(w):
        # [L,768,768] -> [L, 128, (pair=3, two=2, dout=768)] fp8 of 64*w
        r = w.reshape(L, NKP, 2, PD, D).transpose(0, 3, 1, 2, 4).reshape(
            L, PD, NKC * D
        )
        return np.ascontiguousarray(
            np.clip(r * WS, -FP8_MAX, FP8_MAX).astype(fp8)
        )

    def bfmt(b):
        return np.ascontiguousarray(
            b.reshape(L, NKC, PD).transpose(2, 0, 1).reshape(PD, L * NKC)
        )

    shared = {
        "wq": wfmt8(g["Wq"]),
        "wk": wfmt8(g["Wk"]),
        "wv": wfmt8(g["Wv"]),
        "wo1": wfmt8(g["Wo1"]),
        "wo2": wfmt_bf(g["Wo2"]),
        "iden": np.eye(PD, dtype=np.float32),
    }
    if flags["use_bq"]:
        shared["bq"] = bfmt(g["bq"])
    if flags["use_bk"]:
        shared["bk"] = bfmt(g["bk"])
    sel2 = np.zeros((2, PD), dtype=np.float32)
    sel2[0, :64] = WS
    sel2[1, 64:] = WS
    shared["sel2"] = sel2
    if flags["use_b1"]:
        shared["b1bc"] = np.ascontiguousarray(
            np.broadcast_to(b1[:, None, :], (L, PD, D))
        )
    if flags["use_b2"]:
        shared["b2bc"] = np.ascontiguousarray(
            np.broadcast_to(b2[:, None, :], (L, PD, D))
        )
    if flags["use_ln1"]:
        shared["ln1wbc"] = np.ascontiguousarray(
            np.broadcast_to(g["ln1_w"][:, None, :], (L, PD, D))
        )
        shared["ln1bbc"] = np.ascontiguousarray(
            np.broadcast_to(g["ln1_b"][:, None, :], (L, PD, D))
        )
    if flags["use_ln2"]:
        shared["ln2wbc"] = np.ascontiguousarray(
            np.broadcast_to(g["ln2_w"][:, None, :], (L, PD, D))
        )
        shared["ln2bbc"] = np.ascontiguousarray(
            np.broadcast_to(g["ln2_b"][:, None, :], (L, PD, D))
        )

    per_core = []
    for b in range(B):
        m = dict(shared)
        m["qs"] = np.ascontiguousarray(g["query_states"][b])
        m["hs"] = np.ascontiguousarray(g["hidden_states"][b])
        if flags["use_mask"]:
            m["mask"] = np.ascontiguousarray(
                g["attention_mask"][b].reshape(NTC, PD).T
            )
        per_core.append(m)
    return flags, per_core


TRACE = False
LAST_EXEC_NS = None
LAST_RESULTS = None


def kernel(**inputs):
    global LAST_EXEC_NS, LAST_RESULTS
    from concourse.bass_utils import run_bass_kernel_spmd

    flags, per_core = prep_inputs(inputs)
    nc = build_nc(flags)
    kw = {}
    if TRACE:
        kw = dict(trace=True, tmpdir="/root/problem/trace_out")
        import os

        os.makedirs("/root/problem/trace_out", exist_ok=True)
    res = run_bass_kernel_spmd(nc, per_core, core_ids=list(range(B)), **kw)
    LAST_EXEC_NS = res.exec_time_ns
    LAST_RESULTS = res
    out = np.stack([np.asarray(res.results[b]["out"]) for b in range(B)], axis=0)
    return out.astype(np.float32)
